# revision 1
# baseline (speedup 1.0000x reference)
"""HGConv fused kernel for one TRN2 chip (8 NeuronCores), SPMD via Bass/Tile.

Hardcoded for M=16384 nodes, E=4096 hyperedges, D=300, N_CAT=3, 8 cores.

  - Shard the node axis m: core c gets node_feats rows [2048c, 2048(c+1))
    and the matching inc_mat rows.  Phase 1 computes the partial
    IX_c = inc_c.T @ X_c (4096, 300) with inc tiles stationary on the PE.
  - ReduceScatter(add) turns the partials into the true IX = inc.T @ X,
    e-sharded: core c owns edges [512c, 512(c+1)).
  - Local tail per core: edge_att = IX @ W_att (reassociated from
    inc.T @ (X @ W_att)), softmax over d, ef = (IX * attn) @ W_proj,
    residual mix with edge_feats, scores = ef2 @ ec_W_att, locally
    stabilized exp, G = ef2 @ ec_W_proj, partial pooled vector
    p2 = sum_e exp_e * G[e, :].
  - AllGather of the per-core (p2, z, m) partials (304 floats); every core
    redundantly combines them (global softmax over edges) and applies the
    two tiny projections to produce the (3,) logits.
"""

import sys

for _p in ("/opt/trn_rl_repo", "/opt/pypackages"):
    if _p not in sys.path:
        sys.path.append(_p)

import numpy as np

import concourse.bacc as bacc
import concourse.tile as tile
from concourse import masks, mybir
from concourse.bass_utils import run_bass_kernel_spmd

F32 = mybir.dt.float32
F32R = mybir.dt.float32r
BF16 = mybir.dt.bfloat16
AX = mybir.AxisListType
OP = mybir.AluOpType
AF = mybir.ActivationFunctionType

NCORES = 8
M, E, D, NCAT = 16384, 4096, 300, 3
M_SH = M // NCORES          # 2048 nodes per core
E_SH = E // NCORES          # 512 edges per core (tail shard)
MT = M_SH // 128            # 16 m-tiles per core
ET_SH = E_SH // 128         # 4 e-tiles per core
DCH = (128, 128, 44)        # d split into partition chunks
DOF = (0, 128, 256)
E_BLK = 1024                # phase-1 e block (8 psum banks)
N_EBLK = E // E_BLK
E_SUB = E_BLK // 128


def _build(alpha: float, mode: str):
    nc = bacc.Bacc("TRN2", target_bir_lowering=False, debug=False,
                   num_devices=NCORES)
    in_dt = BF16 if mode == "bf16" else F32
    x_d = nc.dram_tensor("x", [M_SH, D], in_dt, kind="ExternalInput")
    inc_d = nc.dram_tensor("inc", [M_SH, E], in_dt, kind="ExternalInput")
    ef_d = nc.dram_tensor("efeat", [E_SH, D], F32, kind="ExternalInput")
    watt_d = nc.dram_tensor("watt", [D, D], F32, kind="ExternalInput")
    wproj_d = nc.dram_tensor("wproj", [D, D], F32, kind="ExternalInput")
    ecwatt_d = nc.dram_tensor("ecwatt", [D, 1], F32, kind="ExternalInput")
    ecwproj_d = nc.dram_tensor("ecwproj", [D, D], F32, kind="ExternalInput")
    ecb_d = nc.dram_tensor("ecb", [D], F32, kind="ExternalInput")
    fcw_d = nc.dram_tensor("fcw", [D, NCAT], F32, kind="ExternalInput")
    fcb_d = nc.dram_tensor("fcb", [NCAT], F32, kind="ExternalInput")
    out_d = nc.dram_tensor("out", [1, NCAT], F32, kind="ExternalOutput")

    groups = [list(range(NCORES))]

    rdt = {"f32": F32, "f32r": F32R, "bf16": BF16}[mode]
    e_blk = 2048 if mode == "bf16" else 1024
    n_eblk = E // e_blk
    e_sub = e_blk // 128

    def mm(out, lhsT, rhs, start, stop):
        nc.tensor.matmul(out, lhsT, rhs, start=start, stop=stop)

    def rsrc(ap):
        return ap.bitcast(F32R) if mode == "f32r" else ap

    with tile.TileContext(nc) as tc, \
         tc.tile_pool(name="sb", bufs=1) as sb, \
         tc.tile_pool(name="dram", bufs=1, space="DRAM") as dram:

        p_chunks = [dram.tile([1024, D], F32, name=f"p_chunk{k}")
                    for k in range(4)]          # RS inputs (partial IX)
        r_ks = [dram.tile([128, D], F32, name=f"r_k{k}")
                for k in range(4)]              # RS outputs (my 128 edges)
        pk_dram = dram.tile([304], F32)         # AG input
        gath = dram.tile([NCORES, 304], F32)    # AG output

        # ---------- phase 1: IX partial = inc_c.T @ X_c ----------
        x_sb = sb.tile([128, MT, D], rdt)
        nc.sync.dma_start(x_sb[:], rsrc(x_d.ap().rearrange("(t p) d -> p t d",
                                                           p=128)))
        with tc.tile_pool(name="incp", bufs=MT + 8) as incp, \
             tc.tile_pool(name="stg", bufs=8) as stg, \
             tc.tile_pool(name="pp1", bufs=8, space="PSUM") as pp1:
            for blk in range(n_eblk):
                inc_sb = [incp.tile([128, e_blk], rdt, tag="inc",
                                    name=f"inc_b{blk}_m{m}")
                          for m in range(MT)]
                for m in range(MT):
                    eng = nc.sync if m % 2 == 0 else nc.scalar
                    eng.dma_start(
                        inc_sb[m][:],
                        rsrc(inc_d[m * 128:(m + 1) * 128,
                                   blk * e_blk:(blk + 1) * e_blk]))
                for es in range(e_sub):
                    acc = pp1.tile([128, D], F32, tag="p1")
                    for m in range(MT):
                        mm(acc[:], inc_sb[m][:, es * 128:(es + 1) * 128],
                           x_sb[:, m, :], start=(m == 0), stop=(m == MT - 1))
                    stage = stg.tile([128, D], F32, tag="stage",
                                     name=f"stage_{blk}_{es}")
                    nc.vector.tensor_copy(stage[:], acc[:])
                    eg = blk * e_blk + es * 128        # global edge offset
                    k, row = eg // 1024, eg % 1024
                    nc.gpsimd.dma_start(p_chunks[k][row:row + 128, :],
                                        stage[:])
                    # phase 2 (chunked, overlapped): as soon as chunk k is
                    # fully written, ReduceScatter it while the next block
                    # computes.
                    if row == 1024 - 128:
                        nc.gpsimd.collective_compute(
                            "ReduceScatter", OP.add, replica_groups=groups,
                            ins=[p_chunks[k].opt()], outs=[r_ks[k].opt()])

        # ---------- small weights / constants ----------
        watt_sb = sb.tile([128, 3, D], F32)
        wproj_sb = sb.tile([128, 3, D], F32)
        ecwproj_sb = sb.tile([128, 3, D], F32)
        fcw_sb = sb.tile([128, 3, NCAT], F32)
        ecwatt_sb = sb.tile([128, 3, 1], F32)
        for i, (c, o) in enumerate(zip(DCH, DOF)):
            nc.sync.dma_start(watt_sb[:c, i, :], watt_d[o:o + c, :])
            nc.sync.dma_start(wproj_sb[:c, i, :], wproj_d[o:o + c, :])
            nc.sync.dma_start(ecwproj_sb[:c, i, :], ecwproj_d[o:o + c, :])
            nc.sync.dma_start(fcw_sb[:c, i, :], fcw_d[o:o + c, :])
            nc.sync.dma_start(ecwatt_sb[:c, i, :], ecwatt_d[o:o + c, :])
        ecb_sb = sb.tile([1, D], F32)
        nc.sync.dma_start(ecb_sb[:], ecb_d.ap().rearrange("(o d) -> o d", o=1))
        fcb_sb = sb.tile([1, NCAT], F32)
        nc.sync.dma_start(fcb_sb[:], fcb_d.ap().rearrange("(o d) -> o d", o=1))
        ident = sb.tile([128, 128], F32)
        masks.make_identity(nc, ident[:])
        efeat_sb = sb.tile([128, ET_SH, D], F32)
        nc.sync.dma_start(efeat_sb[:],
                          ef_d.ap().rearrange("(t p) d -> p t d", p=128))

        # ---------- phase 3: local tail on this core's 512 edges ----------
        ix_sb = sb.tile([128, ET_SH, D], F32)
        for k in range(4):
            nc.sync.dma_start(ix_sb[:, k, :], r_ks[k][:])

        with tc.tile_pool(name="pp2", bufs=4, space="PSUM") as pp:

            def transpose_512xD(src_sb, dstT_sb):
                # src (128, 4, 300) [e-part] -> dstT (128, 3, 512) [d-part]
                for et in range(ET_SH):
                    for i, (c, o) in enumerate(zip(DCH, DOF)):
                        tp = pp.tile([128, 128], F32, tag="ps")
                        nc.tensor.transpose(tp[:c, :128],
                                            src_sb[:, et, o:o + c], ident[:])
                        nc.scalar.copy(
                            dstT_sb[:c, i, et * 128:(et + 1) * 128],
                            tp[:c, :128])

            ixT_sb = sb.tile([128, 3, E_SH], F32)
            transpose_512xD(ix_sb, ixT_sb)

            # edge_att = IX @ W_att; softmax over d; ef = IX * attn
            ef2_sb = sb.tile([128, ET_SH, D], F32)
            stat_sb = sb.tile([128, ET_SH, 4], F32)
            for et in range(ET_SH):
                att = pp.tile([128, D], F32, tag="ps")
                for i, c in enumerate(DCH):
                    mm(att[:], ixT_sb[:c, i, et * 128:(et + 1) * 128],
                       watt_sb[:c, i, :], start=(i == 0), stop=(i == 2))
                nmax = stat_sb[:, et, 0:1]
                nc.vector.tensor_reduce(nmax, att[:], axis=AX.X, op=OP.max,
                                        negate=True)
                ex = pp.tile([128, D], F32, tag="ps")
                rsum = stat_sb[:, et, 1:2]
                nc.scalar.activation(ex[:], att[:], AF.Exp, bias=nmax,
                                     scale=1.0, accum_out=rsum)
                rcp = stat_sb[:, et, 2:3]
                nc.vector.reciprocal(rcp, rsum)
                nc.vector.scalar_tensor_tensor(
                    ef2_sb[:, et, :], ex[:], rcp, ix_sb[:, et, :],
                    op0=OP.mult, op1=OP.mult)

            efT_sb = sb.tile([128, 3, E_SH], F32)
            transpose_512xD(ef2_sb, efT_sb)

            # ef2 = alpha * edge_feats + (1 - alpha) * (ef @ W_proj)
            efs_sb = sb.tile([128, ET_SH, D], F32)
            for et in range(ET_SH):
                prj = pp.tile([128, D], F32, tag="ps")
                for i, c in enumerate(DCH):
                    mm(prj[:], efT_sb[:c, i, et * 128:(et + 1) * 128],
                       wproj_sb[:c, i, :], start=(i == 0), stop=(i == 2))
                nc.scalar.mul(efs_sb[:, et, :], efeat_sb[:, et, :],
                              float(alpha))
                nc.vector.scalar_tensor_tensor(
                    ef2_sb[:, et, :], prj[:], float(1.0 - alpha),
                    efs_sb[:, et, :], op0=OP.mult, op1=OP.add)

            ef2T_sb = sb.tile([128, 3, E_SH], F32)
            transpose_512xD(ef2_sb, ef2T_sb)

            # scores (1, 512); locally stabilized exp weights
            sc = pp.tile([1, E_SH], F32, tag="ps")
            for i, c in enumerate(DCH):
                mm(sc[:], ecwatt_sb[:c, i, :], ef2T_sb[:c, i, :],
                   start=(i == 0), stop=(i == 2))
            one_sb = sb.tile([1, 520], F32)
            nloc = one_sb[:, 512:513]
            nc.vector.tensor_reduce(nloc, sc[:], axis=AX.X, op=OP.max,
                                    negate=True)
            expw = one_sb[:, 0:512]
            zloc = one_sb[:, 513:514]
            nc.scalar.activation(expw, sc[:], AF.Exp, bias=nloc, scale=1.0,
                                 accum_out=zloc)
            mloc = one_sb[:, 514:515]
            nc.scalar.mul(mloc, nloc, -1.0)

            expcol_sb = sb.tile([128, ET_SH], F32)
            for et in range(ET_SH):
                tc1 = pp.tile([128, 1], F32, tag="ps")
                nc.tensor.transpose(tc1[:],
                                    expw[0:1, et * 128:(et + 1) * 128],
                                    ident[0:1, 0:1])
                nc.scalar.copy(expcol_sb[:, et:et + 1], tc1[:])

            # G = ef2 @ ec_W_proj ; p2 = expw^T @ G (pooling + proj folded)
            g_sb = sb.tile([128, ET_SH, D], F32)
            for et in range(ET_SH):
                g = pp.tile([128, D], F32, tag="ps")
                for i, c in enumerate(DCH):
                    mm(g[:], ef2T_sb[:c, i, et * 128:(et + 1) * 128],
                       ecwproj_sb[:c, i, :], start=(i == 0), stop=(i == 2))
                nc.scalar.copy(g_sb[:, et, :], g[:])
            p2 = pp.tile([1, D], F32, tag="acc")
            for et in range(ET_SH):
                mm(p2[:], expcol_sb[:, et:et + 1], g_sb[:, et, :],
                   start=(et == 0), stop=(et == ET_SH - 1))

            pk_sb = sb.tile([1, 304], F32)
            nc.scalar.copy(pk_sb[:, 0:D], p2[:])
            nc.scalar.copy(pk_sb[:, 300:301], zloc)
            nc.scalar.copy(pk_sb[:, 301:302], mloc)
            nc.vector.memset(pk_sb[:, 302:304], 0.0)
            nc.sync.dma_start(pk_dram[:], pk_sb[0:1, :])

            # ---------- phase 4: AllGather + redundant epilogue ----------
            nc.gpsimd.collective_compute(
                "AllGather", OP.bypass, replica_groups=groups,
                ins=[pk_dram.opt()], outs=[gath.opt()])

            grow = sb.tile([1, NCORES, 304], F32)
            nc.sync.dma_start(
                grow[:], gath[:].rearrange("c k -> (c k)").rearrange(
                    "(o c k) -> o c k", o=1, c=NCORES))
            g8 = sb.tile([NCORES, 304], F32)
            nc.sync.dma_start(g8[:], gath[:])

            eps_sb = sb.tile([1, 16], F32)
            ngmax = eps_sb[:, 0:1]
            nc.vector.tensor_reduce(ngmax, grow[:, :, 301], axis=AX.X,
                                    op=OP.max, negate=True)
            scal_row = eps_sb[:, 1:9]
            nc.scalar.activation(scal_row, grow[:, :, 301], AF.Exp,
                                 bias=ngmax, scale=1.0)
            sccol = pp.tile([NCORES, 1], F32, tag="ps")
            nc.tensor.transpose(sccol[:], scal_row, ident[0:1, 0:1])
            sccol_sb = sb.tile([NCORES, 1], F32)
            nc.scalar.copy(sccol_sb[:], sccol[:])
            comb = pp.tile([1, 304], F32, tag="ps")
            nc.tensor.matmul(comb[:], sccol_sb[:], g8[:], start=True,
                             stop=True)
            rz = eps_sb[:, 9:10]
            nc.vector.reciprocal(rz, comb[:, 300:301])
            pooled_sb = sb.tile([1, D], F32)
            nc.vector.tensor_scalar_mul(pooled_sb[:], comb[:, 0:D], rz)
            nc.vector.tensor_add(pooled_sb[:], pooled_sb[:], ecb_sb[:])

            ocol_sb = sb.tile([128, 3], F32)
            for i, (c, o) in enumerate(zip(DCH, DOF)):
                tpc = pp.tile([128, 1], F32, tag="ps")
                nc.tensor.transpose(tpc[:c, :], pooled_sb[0:1, o:o + c],
                                    ident[0:1, 0:1])
                nc.scalar.copy(ocol_sb[:c, i:i + 1], tpc[:c, :])
            lg = pp.tile([1, NCAT], F32, tag="acc")
            for i, c in enumerate(DCH):
                nc.tensor.matmul(lg[:], ocol_sb[:c, i:i + 1],
                                 fcw_sb[:c, i, :], start=(i == 0),
                                 stop=(i == 2))
            logit_sb = sb.tile([1, NCAT], F32)
            nc.vector.tensor_add(logit_sb[:], lg[:], fcb_sb[:])
            nc.sync.dma_start(out_d[:], logit_sb[:])

    nc.compile()
    return nc


_CACHE = {}


def get_nc(alpha: float, mode: str = "f32r"):
    key = (alpha, mode)
    if key not in _CACHE:
        _CACHE[key] = _build(alpha, mode)
    return _CACHE[key]


def make_in_maps(node_feats, edge_feats, inc_mat, W_att, W_proj,
                 ec_W_att, ec_W_proj, ec_b_proj, fc_W, fc_b, mode="f32r"):
    cc = lambda a: np.ascontiguousarray(np.asarray(a, np.float32))
    node_feats, inc_mat, edge_feats = cc(node_feats), cc(inc_mat), cc(edge_feats)
    if mode == "bf16":
        import ml_dtypes
        node_feats = node_feats.astype(ml_dtypes.bfloat16)
        inc_mat = inc_mat.astype(ml_dtypes.bfloat16)
    common = dict(watt=cc(W_att), wproj=cc(W_proj),
                  ecwatt=cc(ec_W_att).reshape(D, 1), ecwproj=cc(ec_W_proj),
                  ecb=cc(ec_b_proj), fcw=cc(fc_W), fcb=cc(fc_b))
    in_maps = []
    for c in range(NCORES):
        # under chunked RS, core c owns edges {1024k + 128c .. +128} k=0..3
        eidx = np.concatenate([np.arange(1024 * k + 128 * c,
                                         1024 * k + 128 * (c + 1))
                               for k in range(4)])
        in_maps.append(dict(
            x=node_feats[c * M_SH:(c + 1) * M_SH],
            inc=np.ascontiguousarray(inc_mat[c * M_SH:(c + 1) * M_SH]),
            efeat=np.ascontiguousarray(edge_feats[eidx]),
            **common))
    return in_maps


def kernel(node_feats, edge_feats, inc_mat, W_att, W_proj, alpha,
           ec_W_att, ec_W_proj, ec_b_proj, fc_W, fc_b,
           mode="f32r", trace=False):
    nc = get_nc(float(np.asarray(alpha)), mode)
    in_maps = make_in_maps(node_feats, edge_feats, inc_mat, W_att, W_proj,
                           ec_W_att, ec_W_proj, ec_b_proj, fc_W, fc_b,
                           mode=mode)
    res = run_bass_kernel_spmd(nc, in_maps, list(range(NCORES)), trace=trace)
    kernel.last_results = res
    return res.results[0]["out"].reshape(NCAT).astype(np.float32)



# revision 8
# speedup vs baseline: 1.3152x; 1.3152x over previous
"""HGConv fused kernel for one TRN2 chip (8 NeuronCores), SPMD via Bass/Tile.

Hardcoded for M=16384 nodes, E=4096 hyperedges, D=300, N_CAT=3, 8 cores.

Edge-sharded design (v2):
  - Core c owns hyperedges [512c, 512(c+1)).  It loads the FULL node
    features X (bf16, tiled) and its 512-column slice of inc (bf16,
    tiled), streaming both in m-blocks, and computes
    IX_c = inc[:, ec].T @ X  (512, 300) entirely locally -- no
    ReduceScatter (the baseline's RS moved 4.9MB/core through a ~31GB/s
    8-rank ring, costing ~150us).
  - Local tail on the 512 edges: att = IX @ W_att (via PE transpose of
    IX), softmax over d, P = IX * attn, ef_p = P @ W_proj,
    ef2 = a*efeat + (1-a)*ef_p.  Edge scores are reassociated:
    s = a*(efeat @ ec_W_att) + (1-a)*(P @ (W_proj @ ec_W_att))
    so no third transpose is needed; the pooled partial is
    p2 = sum_e exp(s_e - m_loc) * ef2[e, :]   (ec_W_proj deferred).
  - One tiny AllGather of (p2, z, m) partials (304 floats); every core
    redundantly combines (global softmax over edges), applies ec_W_proj
    + bias and the final classifier to produce the (3,) logits.
"""

import sys

for _p in ("/opt/trn_rl_repo", "/opt/pypackages"):
    if _p not in sys.path:
        sys.path.append(_p)

import numpy as np

import concourse.bacc as bacc
import concourse.tile as tile
from concourse import masks, mybir
from concourse.bass_utils import run_bass_kernel_spmd

F32 = mybir.dt.float32
F32R = mybir.dt.float32r
BF16 = mybir.dt.bfloat16
AX = mybir.AxisListType
OP = mybir.AluOpType
AF = mybir.ActivationFunctionType

NCORES = 8
M, E, D, NCAT = 16384, 4096, 300, 3
E_SH = E // NCORES          # 512 edges per core
MT = M // 128               # 128 m-tiles (full node axis on every core)
ET = E_SH // 128            # 4 local e-tiles
DCH = (128, 128, 44)        # d split into partition chunks
DOF = (0, 128, 256)
BLK = 8                     # m-tiles per DMA block
NBLK = MT // BLK            # 16 blocks


def _build(alpha: float, mode: str):
    nc = bacc.Bacc("TRN2", target_bir_lowering=False, debug=False,
                   num_devices=NCORES)
    in_dt = BF16 if mode == "bf16" else F32
    a = float(alpha)

    xt_d = nc.dram_tensor("xt", [128, MT, D], in_dt, kind="ExternalInput")
    inct_d = nc.dram_tensor("inct", [128, MT, E_SH], in_dt,
                            kind="ExternalInput")
    ef_d = nc.dram_tensor("efeat", [E_SH, D], F32, kind="ExternalInput")
    eft_d = nc.dram_tensor("efeatt", [D, E_SH], F32, kind="ExternalInput")
    watt_d = nc.dram_tensor("watt", [D, D], F32, kind="ExternalInput")
    wproj_d = nc.dram_tensor("wproj", [D, D], F32, kind="ExternalInput")
    wprojt_d = nc.dram_tensor("wprojt", [D, D], F32, kind="ExternalInput")
    ecwatt_d = nc.dram_tensor("ecwatt", [D, 1], F32, kind="ExternalInput")
    ecwproj_d = nc.dram_tensor("ecwproj", [D, D], F32, kind="ExternalInput")
    ecb_d = nc.dram_tensor("ecb", [D], F32, kind="ExternalInput")
    fcw_d = nc.dram_tensor("fcw", [D, NCAT], F32, kind="ExternalInput")
    fcb_d = nc.dram_tensor("fcb", [NCAT], F32, kind="ExternalInput")
    out_d = nc.dram_tensor("out", [1, NCAT], F32, kind="ExternalOutput")

    groups = [list(range(NCORES))]

    def rsrc(ap):
        return ap.bitcast(F32R) if mode == "f32r" else ap

    def mm(out, lhsT, rhs, start, stop):
        nc.tensor.matmul(out, lhsT, rhs, start=start, stop=stop)

    with tile.TileContext(nc) as tc, \
         tc.tile_pool(name="sb", bufs=1) as sb, \
         tc.tile_pool(name="dram", bufs=1, space="DRAM") as dram:

        pk_dram = dram.tile([304], F32)         # AG input
        gath = dram.tile([NCORES, 304], F32)    # AG output

        # ---------- small weights / constants (early, gpsimd queue) ----
        watt_sb = sb.tile([128, 3, D], F32)
        wproj_sb = sb.tile([128, 3, D], F32)
        wprojt_sb = sb.tile([128, 3, D], F32)
        ecwproj_sb = sb.tile([128, 3, D], F32)
        fcw_sb = sb.tile([128, 3, NCAT], F32)
        ecwatt_sb = sb.tile([128, 3, 1], F32)
        eft_sb = sb.tile([128, 3, E_SH], F32)
        for i, (c, o) in enumerate(zip(DCH, DOF)):
            nc.gpsimd.dma_start(watt_sb[:c, i, :], watt_d[o:o + c, :])
            nc.gpsimd.dma_start(wproj_sb[:c, i, :], wproj_d[o:o + c, :])
            nc.gpsimd.dma_start(wprojt_sb[:c, i, :], wprojt_d[o:o + c, :])
            nc.gpsimd.dma_start(ecwproj_sb[:c, i, :], ecwproj_d[o:o + c, :])
            nc.gpsimd.dma_start(fcw_sb[:c, i, :], fcw_d[o:o + c, :])
            nc.gpsimd.dma_start(ecwatt_sb[:c, i, :], ecwatt_d[o:o + c, :])
            nc.gpsimd.dma_start(eft_sb[:c, i, :], eft_d[o:o + c, :])
        ecb_sb = sb.tile([1, D], F32)
        nc.gpsimd.dma_start(ecb_sb[:], ecb_d.ap().rearrange("(o d) -> o d",
                                                            o=1))
        fcb_sb = sb.tile([1, NCAT], F32)
        nc.gpsimd.dma_start(fcb_sb[:], fcb_d.ap().rearrange("(o d) -> o d",
                                                            o=1))
        efeat_sb = sb.tile([128, ET, D], F32)
        nc.gpsimd.dma_start(efeat_sb[:],
                            ef_d.ap().rearrange("(t p) d -> p t d", p=128))
        ident = sb.tile([128, 128], F32)
        masks.make_identity(nc, ident[:])

        ix_sb = sb.tile([128, ET, D], F32)
        w2col = sb.tile([128, 3, 1], F32)
        sea_row = sb.tile([1, E_SH], F32)
        efs_sb = sb.tile([128, ET, D], F32)

        with tc.tile_pool(name="ppix", bufs=1, space="PSUM") as ppix, \
             tc.tile_pool(name="pp0", bufs=2, space="PSUM") as pp0, \
             tc.tile_pool(name="xp", bufs=4) as xp, \
             tc.tile_pool(name="ip", bufs=4) as ip:

            # ---- precompute (overlaps phase 1; only needs weight DMAs) ----
            # w2 = W_proj @ ec_W_att as a column (d-part):
            #   w2row[1, dc] = sum_d2 ecwatt[d2] * WprojT[d2, dc]
            w2ps = pp0.tile([1, D], F32, tag="ps")
            for i, c in enumerate(DCH):
                mm(w2ps[:], ecwatt_sb[:c, i, :], wprojt_sb[:c, i, :],
                   start=(i == 0), stop=(i == 2))
            w2row = sb.tile([1, D], F32)
            nc.scalar.copy(w2row[:], w2ps[:])
            for i, (c, o) in enumerate(zip(DCH, DOF)):
                tpc = pp0.tile([128, 1], F32, tag="ps")
                nc.tensor.transpose(tpc[:c, :], w2row[0:1, o:o + c],
                                    ident[0:1, 0:1])
                nc.scalar.copy(w2col[:c, i, :], tpc[:c, :])
            # sE = efeat @ ec_W_att as a row, pre-scaled by alpha
            seps = pp0.tile([1, E_SH], F32, tag="ps")
            for i, c in enumerate(DCH):
                mm(seps[:], ecwatt_sb[:c, i, :], eft_sb[:c, i, :],
                   start=(i == 0), stop=(i == 2))
            nc.scalar.mul(sea_row[:], seps[:], a)
            # efs = alpha * efeat
            nc.scalar.mul(efs_sb[:], efeat_sb[:], a)

            # ---------- phase 1: IX = inc_cols.T @ X over all m ----------
            ixps = [ppix.tile([128, D], F32, tag=f"ix{ec}", name=f"ix{ec}")
                    for ec in range(ET)]
            for b in range(NBLK):
                xb = xp.tile([128, BLK, D], in_dt, tag="xb", name=f"xb{b}")
                nc.sync.dma_start(xb[:], rsrc(xt_d[:, b * BLK:(b + 1) * BLK, :]))
                ib = ip.tile([128, BLK, E_SH], in_dt, tag="ib", name=f"ib{b}")
                nc.scalar.dma_start(ib[:],
                                    rsrc(inct_d[:, b * BLK:(b + 1) * BLK, :]))
                for tl in range(BLK):
                    mt = b * BLK + tl
                    for ec in range(ET):
                        mm(ixps[ec][:], ib[:, tl, ec * 128:(ec + 1) * 128],
                           xb[:, tl, :], start=(mt == 0), stop=(mt == MT - 1))

            # ---------- evacuate IX ----------
            for et in range(ET):
                if et % 2 == 0:
                    nc.vector.tensor_copy(ix_sb[:, et, :], ixps[et][:])
                else:
                    nc.scalar.copy(ix_sb[:, et, :], ixps[et][:])

        # ---------- tail on this core's 512 edges ----------
        with tc.tile_pool(name="pp", bufs=6, space="PSUM") as pp, \
             tc.tile_pool(name="ppa", bufs=1, space="PSUM") as ppa:

            def transpose_512xD(src_sb, dstT_sb):
                # src (128, 4, 300) [e-part] -> dstT (128, 3, 512) [d-part]
                for et in range(ET):
                    for i, (c, o) in enumerate(zip(DCH, DOF)):
                        tp = pp.tile([128, 128], F32, tag="ps")
                        nc.tensor.transpose(tp[:c, :128],
                                            src_sb[:, et, o:o + c], ident[:])
                        nc.scalar.copy(
                            dstT_sb[:c, i, et * 128:(et + 1) * 128],
                            tp[:c, :128])

            ixT_sb = sb.tile([128, 3, E_SH], F32)
            transpose_512xD(ix_sb, ixT_sb)

            # att = IX @ W_att; softmax over d; P = IX * attn
            p_sb = sb.tile([128, ET, D], F32)
            stat_sb = sb.tile([128, ET, 4], F32)
            for et in range(ET):
                att = pp.tile([128, D], F32, tag="ps")
                for i, c in enumerate(DCH):
                    mm(att[:], ixT_sb[:c, i, et * 128:(et + 1) * 128],
                       watt_sb[:c, i, :], start=(i == 0), stop=(i == 2))
                nmax = stat_sb[:, et, 0:1]
                nc.vector.tensor_reduce(nmax, att[:], axis=AX.X, op=OP.max,
                                        negate=True)
                ex = pp.tile([128, D], F32, tag="ps")
                rsum = stat_sb[:, et, 1:2]
                nc.scalar.activation(ex[:], att[:], AF.Exp, bias=nmax,
                                     scale=1.0, accum_out=rsum)
                rcp = stat_sb[:, et, 2:3]
                nc.vector.reciprocal(rcp, rsum)
                nc.vector.scalar_tensor_tensor(
                    p_sb[:, et, :], ex[:], rcp, ix_sb[:, et, :],
                    op0=OP.mult, op1=OP.mult)

            pT_sb = sb.tile([128, 3, E_SH], F32)
            transpose_512xD(p_sb, pT_sb)

            # ef2 = alpha*efeat + (1-alpha) * (P @ W_proj)
            ef2_sb = sb.tile([128, ET, D], F32)
            for et in range(ET):
                prj = pp.tile([128, D], F32, tag="ps")
                for i, c in enumerate(DCH):
                    mm(prj[:], pT_sb[:c, i, et * 128:(et + 1) * 128],
                       wproj_sb[:c, i, :], start=(i == 0), stop=(i == 2))
                nc.vector.scalar_tensor_tensor(
                    ef2_sb[:, et, :], prj[:], float(1.0 - a),
                    efs_sb[:, et, :], op0=OP.mult, op1=OP.add)

            # s = a*sE + (1-a)*(P @ w2); locally stabilized exp weights
            sps = pp.tile([1, E_SH], F32, tag="ps")
            for i, c in enumerate(DCH):
                mm(sps[:], w2col[:c, i, :], pT_sb[:c, i, :],
                   start=(i == 0), stop=(i == 2))
            one_sb = sb.tile([1, 520], F32)
            s_row = one_sb[:, 0:512]
            nc.vector.scalar_tensor_tensor(s_row, sps[:], float(1.0 - a),
                                           sea_row[:], op0=OP.mult,
                                           op1=OP.add)
            nloc = one_sb[:, 512:513]
            nc.vector.tensor_reduce(nloc, s_row, axis=AX.X, op=OP.max,
                                    negate=True)
            expw_sb = sb.tile([1, E_SH], F32)
            zloc = one_sb[:, 513:514]
            nc.scalar.activation(expw_sb[:], s_row, AF.Exp, bias=nloc,
                                 scale=1.0, accum_out=zloc)
            mloc = one_sb[:, 514:515]
            nc.scalar.mul(mloc, nloc, -1.0)

            expcol_sb = sb.tile([128, ET], F32)
            for et in range(ET):
                tc1 = pp.tile([128, 1], F32, tag="ps")
                nc.tensor.transpose(tc1[:],
                                    expw_sb[0:1, et * 128:(et + 1) * 128],
                                    ident[0:1, 0:1])
                nc.scalar.copy(expcol_sb[:, et:et + 1], tc1[:])

            # p2 = sum_e expw_e * ef2[e, :]   (ec_W_proj deferred)
            p2 = ppa.tile([1, D], F32, tag="acc")
            for et in range(ET):
                mm(p2[:], expcol_sb[:, et:et + 1], ef2_sb[:, et, :],
                   start=(et == 0), stop=(et == ET - 1))

            pk_sb = sb.tile([1, 304], F32)
            nc.scalar.copy(pk_sb[:, 0:D], p2[:])
            nc.scalar.copy(pk_sb[:, 300:301], zloc)
            nc.scalar.copy(pk_sb[:, 301:302], mloc)
            nc.vector.memset(pk_sb[:, 302:304], 0.0)
            nc.sync.dma_start(pk_dram[:], pk_sb[0:1, :])

            # ---------- AllGather + redundant epilogue ----------
            nc.gpsimd.collective_compute(
                "AllGather", OP.bypass, replica_groups=groups,
                ins=[pk_dram.opt()], outs=[gath.opt()])

            grow = sb.tile([1, NCORES, 304], F32)
            nc.sync.dma_start(
                grow[:], gath[:].rearrange("c k -> (c k)").rearrange(
                    "(o c k) -> o c k", o=1, c=NCORES))
            g8 = sb.tile([NCORES, 304], F32)
            nc.sync.dma_start(g8[:], gath[:])

            eps_sb = sb.tile([1, 16], F32)
            ngmax = eps_sb[:, 0:1]
            nc.vector.tensor_reduce(ngmax, grow[:, :, 301], axis=AX.X,
                                    op=OP.max, negate=True)
            scal_row = eps_sb[:, 1:9]
            nc.scalar.activation(scal_row, grow[:, :, 301], AF.Exp,
                                 bias=ngmax, scale=1.0)
            sccol = pp.tile([NCORES, 1], F32, tag="ps")
            nc.tensor.transpose(sccol[:], scal_row, ident[0:1, 0:1])
            sccol_sb = sb.tile([NCORES, 1], F32)
            nc.scalar.copy(sccol_sb[:], sccol[:])
            comb = pp.tile([1, 304], F32, tag="ps")
            nc.tensor.matmul(comb[:], sccol_sb[:], g8[:], start=True,
                             stop=True)
            rz = eps_sb[:, 9:10]
            nc.vector.reciprocal(rz, comb[:, 300:301])
            pooled_sb = sb.tile([1, D], F32)
            nc.vector.tensor_scalar_mul(pooled_sb[:], comb[:, 0:D], rz)

            # out = pooled @ ec_W_proj + ec_b
            pcol_sb = sb.tile([128, 3, 1], F32)
            for i, (c, o) in enumerate(zip(DCH, DOF)):
                tpc = pp.tile([128, 1], F32, tag="ps")
                nc.tensor.transpose(tpc[:c, :], pooled_sb[0:1, o:o + c],
                                    ident[0:1, 0:1])
                nc.scalar.copy(pcol_sb[:c, i, :], tpc[:c, :])
            o2ps = pp.tile([1, D], F32, tag="ps")
            for i, c in enumerate(DCH):
                mm(o2ps[:], pcol_sb[:c, i, :], ecwproj_sb[:c, i, :],
                   start=(i == 0), stop=(i == 2))
            out2_sb = sb.tile([1, D], F32)
            nc.vector.tensor_add(out2_sb[:], o2ps[:], ecb_sb[:])

            # logits = out @ fc_W + fc_b
            ocol_sb = sb.tile([128, 3, 1], F32)
            for i, (c, o) in enumerate(zip(DCH, DOF)):
                tpc = pp.tile([128, 1], F32, tag="ps")
                nc.tensor.transpose(tpc[:c, :], out2_sb[0:1, o:o + c],
                                    ident[0:1, 0:1])
                nc.scalar.copy(ocol_sb[:c, i, :], tpc[:c, :])
            lg = ppa.tile([1, NCAT], F32, tag="acc")
            for i, c in enumerate(DCH):
                nc.tensor.matmul(lg[:], ocol_sb[:c, i, :],
                                 fcw_sb[:c, i, :], start=(i == 0),
                                 stop=(i == 2))
            logit_sb = sb.tile([1, NCAT], F32)
            nc.vector.tensor_add(logit_sb[:], lg[:], fcb_sb[:])
            nc.sync.dma_start(out_d[:], logit_sb[:])

    nc.compile()
    return nc


_CACHE = {}


def get_nc(alpha: float, mode: str = "bf16"):
    key = (alpha, mode)
    if key not in _CACHE:
        _CACHE[key] = _build(alpha, mode)
    return _CACHE[key]


def _tile_pm(arr2d):
    """(M, K) -> (128, M//128, K) with out[p, t, :] = arr[t*128 + p, :]."""
    mtot, k = arr2d.shape
    return np.ascontiguousarray(
        arr2d.reshape(mtot // 128, 128, k).swapaxes(0, 1))


def make_in_maps(node_feats, edge_feats, inc_mat, W_att, W_proj,
                 ec_W_att, ec_W_proj, ec_b_proj, fc_W, fc_b, mode="bf16"):
    cc = lambda x: np.ascontiguousarray(np.asarray(x, np.float32))
    X = np.asarray(node_feats, np.float32)
    INC = np.asarray(inc_mat, np.float32)
    EF = np.asarray(edge_feats, np.float32)
    if mode == "bf16":
        import ml_dtypes
        X = X.astype(ml_dtypes.bfloat16)
        INC = INC.astype(ml_dtypes.bfloat16)
    xt = _tile_pm(X)
    common = dict(xt=xt, watt=cc(W_att), wproj=cc(W_proj),
                  wprojt=cc(np.asarray(W_proj).T),
                  ecwatt=cc(ec_W_att).reshape(D, 1), ecwproj=cc(ec_W_proj),
                  ecb=cc(ec_b_proj), fcw=cc(fc_W), fcb=cc(fc_b))
    in_maps = []
    for c in range(NCORES):
        ef_sl = np.ascontiguousarray(EF[c * E_SH:(c + 1) * E_SH])
        in_maps.append(dict(
            inct=_tile_pm(INC[:, c * E_SH:(c + 1) * E_SH]),
            efeat=ef_sl,
            efeatt=np.ascontiguousarray(ef_sl.T),
            **common))
    return in_maps


def kernel(node_feats, edge_feats, inc_mat, W_att, W_proj, alpha,
           ec_W_att, ec_W_proj, ec_b_proj, fc_W, fc_b,
           mode="bf16", trace=False):
    nc = get_nc(float(np.asarray(alpha)), mode)
    in_maps = make_in_maps(node_feats, edge_feats, inc_mat, W_att, W_proj,
                           ec_W_att, ec_W_proj, ec_b_proj, fc_W, fc_b,
                           mode=mode)
    res = run_bass_kernel_spmd(nc, in_maps, list(range(NCORES)), trace=trace)
    kernel.last_results = res
    return res.results[0]["out"].reshape(NCAT).astype(np.float32)
